# revision 9
# baseline (speedup 1.0000x reference)
"""Bass/Trainium2 kernel for additive (Bahdanau-style) multi-head attention.

Reference computation (B=2, S=512, D=512, H=8, HD=64):
    q = heads(query @ Wq + bq); k = heads(key_ @ Wk + bk); v = heads(value @ Wv + bv)
    scores[b,h,i,j] = sum_d tanh((q @ Aq)[b,h,i,d] + (k @ Ak)[b,h,j,d]) * av[d]
    attn = softmax(scores, -1); ctx = attn @ v; out = merge(ctx) @ Wo + bo
    returns (out, attn)

Sharding: 8 cores; core c handles batch b = c // 4 and head pair
h0 = 2*(c % 4), h0+1.  Each core computes its two heads' attention and a
partial output projection; the host sums the 4 partial outputs per batch.

Per-core device plan (v2 — ScalarE tanh streaming is the roofline):
  - Projections on TensorE (fp32, exact): qT2/kT2 [128=(h,hd), 512] then
    block-diag Aq/Ak matmul -> qAT2/kAT2 [128, 512(tokens)].
  - The [dd=(h,hd), q, k] pre-tanh sum qAT2[:,q] + kAT2[:,k] is built by two
    engines in parallel (ScalarE must not do it per-q; that serializes at
    (224+512) cycles per query):
      * DVE blocks (11 q each): one tensor_tensor add with stride-0 broadcast
        APs -> [128, 11*512] fp32, then one big ACTIVATE Tanh -> bf16.
      * PE blocks (2 q each): identity-matmul broadcast-adds accumulate
        qcol + k into PSUM [128, 2*512] (bf16 args), then one ACTIVATE Tanh
        reading PSUM -> bf16.
  - TensorE reduces over d with av (bf16): lhsT is a shifted view into a
    [128, 256] buffer whose columns 128/129 hold av for head0/head1 rows, so
    query q accumulates into PSUM rows (2*(q%64), 2*(q%64)+1) of a [128, 512]
    score bank; lhsT width is trimmed to 2*(q%64)+2 columns to cut LDWEIGHTS.
  - Softmax per row: DVE max (negated), ScalarE Exp(bias=-max, accum_out)
    reading PSUM, DVE reciprocal + per-partition multiply.
  - attn rows DMA out interleaved; host de-interleaves.
  - PE transpose of attn tiles -> attnT [k, (group,query,head)] for the
    context matmul; ctx^T [(h,hd), q] accumulated in PSUM (+bv per partition,
    exact because softmax rows sum to 1); output projection vs Wo row-slice
    (+bo/4 so the host-side sum of 4 partials reconstructs bo).
"""

import numpy as np

import concourse.bass as bass
import concourse.mybir as mybir
import concourse.tile as tile
from concourse import bacc
from concourse.bass_utils import run_bass_kernel_spmd
from concourse.masks import make_identity

F32 = mybir.dt.float32
BF16 = mybir.dt.bfloat16
RED_DT = BF16  # dtype of tanh output + av weights for the d-reduction matmul
AF = mybir.ActivationFunctionType

B, S, D, H = 2, 512, 512, 8
HD = D // H  # 64
HP = 2  # heads per core
NCORES = 8
G = S // 64  # score groups of 64 queries -> 8

# per 64-query group: DVE broadcast-add blocks cover the first 56 queries,
# the last 8 run the fused ACTIVATE(bias) path directly on ScalarE
DVE_SPANS = (16, 16, 16, 8)
FUSED_Q = 8
assert sum(DVE_SPANS) + FUSED_Q == 64


def build_nc():
    nc = bacc.Bacc("TRN2", target_bir_lowering=False, debug=False, num_devices=NCORES)

    # ---- DRAM I/O (per-core shards; same names on every core) ----
    xqT = nc.dram_tensor("xqT", [D, S], F32, kind="ExternalInput")
    xkT = nc.dram_tensor("xkT", [D, S], F32, kind="ExternalInput")
    xvT = nc.dram_tensor("xvT", [D, S], F32, kind="ExternalInput")
    wq2 = nc.dram_tensor("wq2", [D, 128], F32, kind="ExternalInput")
    wk2 = nc.dram_tensor("wk2", [D, 128], F32, kind="ExternalInput")
    wv2 = nc.dram_tensor("wv2", [D, 128], F32, kind="ExternalInput")
    wo2 = nc.dram_tensor("wo2", [128, D], F32, kind="ExternalInput")
    bq2 = nc.dram_tensor("bq2", [128, 1], F32, kind="ExternalInput")
    bk2 = nc.dram_tensor("bk2", [128, 1], F32, kind="ExternalInput")
    bv2 = nc.dram_tensor("bv2", [128, 1], F32, kind="ExternalInput")
    bo4 = nc.dram_tensor("bo4", [1, D], F32, kind="ExternalInput")
    aq = nc.dram_tensor("aq", [HD, HD], F32, kind="ExternalInput")
    ak = nc.dram_tensor("ak", [HD, HD], F32, kind="ExternalInput")
    av2 = nc.dram_tensor("av2", [HD, 1], F32, kind="ExternalInput")
    attn_out = nc.dram_tensor("attn_out", [2 * S, S], F32, kind="ExternalOutput")
    out_part = nc.dram_tensor("out_part", [S, D], F32, kind="ExternalOutput")

    with tile.TileContext(nc) as tc:
        with (
            tc.tile_pool(name="const", bufs=1) as const,
            tc.tile_pool(name="tpool", bufs=2) as tpool,
            tc.tile_pool(name="apool", bufs=2) as apool,
            tc.tile_pool(name="stats", bufs=4) as stats,
        ):
            # ---------- load inputs ----------
            xq_t = []
            xk_t = []
            xv_t = []
            for m in range(4):
                t = const.tile([128, S], F32, name=f"xq_{m}")
                nc.sync.dma_start(t[:], xqT[m * 128 : (m + 1) * 128, :])
                xq_t.append(t)
                t = const.tile([128, S], F32, name=f"xk_{m}")
                nc.sync.dma_start(t[:], xkT[m * 128 : (m + 1) * 128, :])
                xk_t.append(t)
                t = const.tile([128, S], F32, name=f"xv_{m}")
                nc.sync.dma_start(t[:], xvT[m * 128 : (m + 1) * 128, :])
                xv_t.append(t)
            wq_t = []
            wk_t = []
            wv_t = []
            for m in range(4):
                t = const.tile([128, 128], F32, name=f"wq_{m}")
                nc.sync.dma_start(t[:], wq2[m * 128 : (m + 1) * 128, :])
                wq_t.append(t)
                t = const.tile([128, 128], F32, name=f"wk_{m}")
                nc.sync.dma_start(t[:], wk2[m * 128 : (m + 1) * 128, :])
                wk_t.append(t)
                t = const.tile([128, 128], F32, name=f"wv_{m}")
                nc.sync.dma_start(t[:], wv2[m * 128 : (m + 1) * 128, :])
                wv_t.append(t)
            wo_t = const.tile([128, D], F32, name="wo_t")
            nc.sync.dma_start(wo_t[:], wo2[:, :])
            bq_t = const.tile([128, 1], F32, name="bq_t")
            nc.sync.dma_start(bq_t[:], bq2[:, :])
            bk_t = const.tile([128, 1], F32, name="bk_t")
            nc.sync.dma_start(bk_t[:], bk2[:, :])
            bv_t = const.tile([128, 1], F32, name="bv_t")
            nc.sync.dma_start(bv_t[:], bv2[:, :])
            # bo/4 replicated across partitions via stride-0 DMA
            bo_rep = const.tile([128, D], F32, name="bo_rep")
            bo_bcast = bass.AP(tensor=bo4.ap().tensor, offset=0, ap=[[0, 128], [1, D]])
            nc.sync.dma_start(bo_rep[:], bo_bcast)

            # block-diagonal Aq/Ak [128, 128]
            aq2 = const.tile([128, 128], F32, name="aq2")
            nc.vector.memset(aq2[:], 0.0)
            nc.sync.dma_start(aq2[0:HD, 0:HD], aq[:, :])
            nc.sync.dma_start(aq2[HD:128, HD:128], aq[:, :])
            ak2 = const.tile([128, 128], F32, name="ak2")
            nc.vector.memset(ak2[:], 0.0)
            nc.sync.dma_start(ak2[0:HD, 0:HD], ak[:, :])
            nc.sync.dma_start(ak2[HD:128, HD:128], ak[:, :])

            # shifted-av buffer: col 128 = av on head0 rows, col 129 on head1
            av_sb = const.tile([128, 1], F32, name="av_sb")
            nc.sync.dma_start(av_sb[0:HD, :], av2[:, :])
            nc.sync.dma_start(av_sb[HD:128, :], av2[:, :])
            avb = const.tile([128, 256], RED_DT, name="avb")
            nc.vector.memset(avb[:], 0.0)
            nc.vector.tensor_copy(avb[0:HD, 128:129], av_sb[0:HD, :])
            nc.vector.tensor_copy(avb[HD:128, 129:130], av_sb[HD:128, :])

            ident = const.tile([128, 128], F32, name="ident")
            make_identity(nc, ident[:])

            # ---------- projections (own PSUM phase) ----------
            qat2 = const.tile([128, S], F32, name="qat2")
            kat2 = const.tile([128, S], F32, name="kat2")
            v_t = []
            with tc.tile_pool(name="ps_pro", bufs=2, space="PSUM") as ps_pro:
                for x_t, w_t, b_t, a2, outT in (
                    (xq_t, wq_t, bq_t, aq2, qat2),
                    (xk_t, wk_t, bk_t, ak2, kat2),
                ):
                    pp = ps_pro.tile([128, S], F32, tag="mm", name="pp")
                    for m in range(4):
                        nc.tensor.matmul(
                            pp[:], w_t[m][:], x_t[m][:], start=(m == 0), stop=(m == 3)
                        )
                    pb = const.tile([128, S], F32, name="pb")
                    nc.vector.tensor_scalar_add(pb[:], pp[:], b_t[:])
                    pa = ps_pro.tile([128, S], F32, tag="mm", name="pa")
                    nc.tensor.matmul(pa[:], a2[:], pb[:], start=True, stop=True)
                    nc.vector.tensor_copy(outT[:], pa[:])

                # v [k, (h,hd)] as 4 tiles of [128, 128]
                for kc in range(4):
                    pv = ps_pro.tile([128, S], F32, tag="mm", name="pv")
                    for m in range(4):
                        nc.tensor.matmul(
                            pv[:, 0:128],
                            xv_t[m][:, kc * 128 : (kc + 1) * 128],
                            wv_t[m][:],
                            start=(m == 0),
                            stop=(m == 3),
                        )
                    vt = const.tile([128, 128], F32, name=f"v_{kc}")
                    nc.vector.tensor_copy(vt[:], pv[:, 0:128])
                    v_t.append(vt)

            # attnT chunks: [k-chunk partitions, (group, query, head) columns]
            attnT = [
                const.tile([128, 2 * S], F32, name=f"attnT_{c}") for c in range(4)
            ]

            # ---------- main loop: scores + softmax + transpose ----------
            with (
                tc.tile_pool(name="ps_sc", bufs=2, space="PSUM") as ps_sc,
                tc.tile_pool(name="ps_tp", bufs=3, space="PSUM") as ps_tp,
            ):
                for g in range(G):
                    sc_ps = ps_sc.tile([128, S], F32, tag="sc", name="sc_ps")

                    def emit_red(i, rhs):
                        if i == 0:
                            # full width: start=True must clear the whole bank
                            # (zero lhsT columns write zeros to rows 2..127)
                            nc.tensor.matmul(
                                sc_ps[:, :], avb[:, 128:256], rhs,
                                start=True, stop=False, skip_group_check=True,
                            )
                        else:
                            nc.tensor.matmul(
                                sc_ps[0 : 2 * i + 2, :],
                                avb[:, 128 - 2 * i : 130],
                                rhs,
                                start=False,
                                stop=(i == 63),
                                skip_group_check=True,
                            )

                    pending = None  # reductions for the previous block
                    i0 = 0
                    # DVE broadcast-add blocks + big-FD tanh
                    for bs in DVE_SPANS:
                        q0 = g * 64 + i0
                        tpre = tpool.tile([128, bs, S], F32, tag="tpre", name="tpre")
                        in0 = (
                            qat2[:, q0 : q0 + bs]
                            .unsqueeze(2)
                            .broadcast_to([128, bs, S])
                        )
                        in1 = kat2[:].unsqueeze(1).broadcast_to([128, bs, S])
                        nc.vector.tensor_add(tpre[:, 0:bs, :], in0, in1)
                        td = tpool.tile([128, bs, S], RED_DT, tag="td", name="td")
                        nc.scalar.activation(td[:, 0:bs, :], tpre[:, 0:bs, :], AF.Tanh)
                        if pending is not None:
                            pending()
                        pending = (
                            lambda td=td, i0=i0, bs=bs: [
                                emit_red(i0 + j, td[:, j, :]) for j in range(bs)
                            ]
                        )
                        i0 += bs
                    pending()
                    # fused ACTIVATE(bias) path for the last FUSED_Q queries
                    for j in range(FUSED_Q):
                        i = 64 - FUSED_Q + j
                        q = g * 64 + i
                        tt = tpool.tile([128, S], RED_DT, tag="tt", name="tt",
                                        bufs=3)
                        nc.scalar.activation(
                            tt[:], kat2[:], AF.Tanh, bias=qat2[:, q : q + 1]
                        )
                        emit_red(i, tt[:])

                    # softmax over the 512 free-dim entries of each (q, h) row
                    mx = stats.tile([128, 1], F32, tag="mx", name="mx")
                    nc.vector.tensor_reduce(
                        mx[:], sc_ps[:], axis=mybir.AxisListType.X,
                        op=mybir.AluOpType.max, negate=True,
                    )
                    esum = stats.tile([128, 1], F32, tag="esum", name="esum")
                    attn_e = apool.tile([128, S], F32, tag="attn_e", name="attn_e")
                    nc.scalar.activation(
                        attn_e[:], sc_ps[:], AF.Exp, bias=mx[:], accum_out=esum[:]
                    )
                    rec = stats.tile([128, 1], F32, tag="rec", name="rec")
                    nc.vector.reciprocal(rec[:], esum[:])
                    attn_n = apool.tile([128, S], F32, tag="attn_n", name="attn_n")
                    nc.vector.tensor_scalar_mul(attn_n[:], attn_e[:], rec[:])
                    nc.sync.dma_start(attn_out[g * 128 : (g + 1) * 128, :], attn_n[:])
                    for c in range(4):
                        tp = ps_tp.tile([128, 128], F32, tag="tp", name="tp")
                        nc.tensor.transpose(
                            tp[:], attn_n[:, c * 128 : (c + 1) * 128], ident[:]
                        )
                        nc.vector.tensor_copy(
                            attnT[c][:, g * 128 : (g + 1) * 128], tp[:]
                        )

            # ---------- context + output projection (own PSUM phase) ----------
            with tc.tile_pool(name="ps_epi", bufs=2, space="PSUM") as ps_epi:
                ctx_ps = ps_epi.tile([128, S], F32, tag="mm", name="ctx_ps")
                for h in range(HP):
                    for c in range(4):
                        rhs = attnT[c][:].rearrange(
                            "p (g i h) -> p g i h", g=G, i=64, h=HP
                        )[:, :, :, h]
                        nc.tensor.matmul(
                            ctx_ps[h * HD : (h + 1) * HD, :],
                            v_t[c][:, h * HD : (h + 1) * HD],
                            rhs,
                            start=(c == 0),
                            stop=(c == 3),
                            skip_group_check=True,
                        )
                ctxT = const.tile([128, S], F32, name="ctxT")
                nc.vector.tensor_scalar_add(ctxT[:], ctx_ps[:], bv_t[:])

                for sc in range(4):
                    op_ps = ps_epi.tile([128, S], F32, tag="mm", name="op_ps")
                    nc.tensor.matmul(
                        op_ps[:], ctxT[:, sc * 128 : (sc + 1) * 128], wo_t[:],
                        start=True, stop=True,
                    )
                    ob = apool.tile([128, S], F32, tag="ob", name="ob")
                    nc.vector.tensor_add(ob[:], op_ps[:], bo_rep[:])
                    nc.sync.dma_start(out_part[sc * 128 : (sc + 1) * 128, :], ob[:])

    nc.compile()
    return nc


_NC_CACHE = None


def _get_nc():
    global _NC_CACHE
    if _NC_CACHE is None:
        _NC_CACHE = build_nc()
    return _NC_CACHE


def _prep_core_inputs(c, query, key_, value, Wq, bq, Wk, bk, Wv, bv, Wo, bo, Aq, Ak, av):
    b = c // 4
    hp = c % 4
    cols = slice(hp * 128, hp * 128 + 128)
    cc = np.ascontiguousarray
    return {
        "xqT": cc(query[b].T),
        "xkT": cc(key_[b].T),
        "xvT": cc(value[b].T),
        "wq2": cc(Wq[:, cols]),
        "wk2": cc(Wk[:, cols]),
        "wv2": cc(Wv[:, cols]),
        "wo2": cc(Wo[cols, :]),
        "bq2": cc(bq[cols][:, None]),
        "bk2": cc(bk[cols][:, None]),
        "bv2": cc(bv[cols][:, None]),
        "bo4": cc((bo * 0.25)[None, :]),
        "aq": cc(Aq),
        "ak": cc(Ak),
        "av2": cc(av[:, None]),
    }


def kernel(**inputs):
    f = lambda name: np.asarray(inputs[name], dtype=np.float32)
    args = (
        f("query"), f("key_"), f("value"),
        f("Wq"), f("bq"), f("Wk"), f("bk"), f("Wv"), f("bv"),
        f("Wo"), f("bo"), f("Aq"), f("Ak"), f("av"),
    )
    nc = _get_nc()
    in_maps = [_prep_core_inputs(c, *args) for c in range(NCORES)]
    res = run_bass_kernel_spmd(nc, in_maps, core_ids=list(range(NCORES)))
    results = res.results

    attn = np.empty((B, H, S, S), dtype=np.float32)
    out = np.zeros((B, S, D), dtype=np.float32)
    for c in range(NCORES):
        b = c // 4
        hp = c % 4
        a = results[c]["attn_out"]  # [1024, 512] rows = (g, i, h) interleaved
        a = a.reshape(G, 64, HP, S).transpose(2, 0, 1, 3).reshape(HP, S, S)
        attn[b, 2 * hp : 2 * hp + 2] = a
        out[b] += results[c]["out_part"]
    return out, attn


# revision 10
# speedup vs baseline: 1.2205x; 1.2205x over previous
"""Bass/Trainium2 kernel for additive (Bahdanau-style) multi-head attention.

Reference computation (B=2, S=512, D=512, H=8, HD=64):
    q = heads(query @ Wq + bq); k = heads(key_ @ Wk + bk); v = heads(value @ Wv + bv)
    scores[b,h,i,j] = sum_d tanh((q @ Aq)[b,h,i,d] + (k @ Ak)[b,h,j,d]) * av[d]
    attn = softmax(scores, -1); ctx = attn @ v; out = merge(ctx) @ Wo + bo
    returns (out, attn)

Sharding: 8 cores; core c handles batch b = c // 4 and head pair
h0 = 2*(c % 4), h0+1.  Each core computes its two heads' attention and a
partial output projection; the host sums the 4 partial outputs per batch.

Per-core device plan (v2 — ScalarE tanh streaming is the roofline):
  - Projections on TensorE (fp32, exact): qT2/kT2 [128=(h,hd), 512] then
    block-diag Aq/Ak matmul -> qAT2/kAT2 [128, 512(tokens)].
  - The [dd=(h,hd), q, k] pre-tanh sum qAT2[:,q] + kAT2[:,k] is built by two
    engines in parallel (ScalarE must not do it per-q; that serializes at
    (224+512) cycles per query):
      * DVE blocks (11 q each): one tensor_tensor add with stride-0 broadcast
        APs -> [128, 11*512] fp32, then one big ACTIVATE Tanh -> bf16.
      * PE blocks (2 q each): identity-matmul broadcast-adds accumulate
        qcol + k into PSUM [128, 2*512] (bf16 args), then one ACTIVATE Tanh
        reading PSUM -> bf16.
  - TensorE reduces over d with av (bf16): lhsT is a shifted view into a
    [128, 256] buffer whose columns 128/129 hold av for head0/head1 rows, so
    query q accumulates into PSUM rows (2*(q%64), 2*(q%64)+1) of a [128, 512]
    score bank; lhsT width is trimmed to 2*(q%64)+2 columns to cut LDWEIGHTS.
  - Softmax per row: DVE max (negated), ScalarE Exp(bias=-max, accum_out)
    reading PSUM, DVE reciprocal + per-partition multiply.
  - attn rows DMA out interleaved; host de-interleaves.
  - PE transpose of attn tiles -> attnT [k, (group,query,head)] for the
    context matmul; ctx^T [(h,hd), q] accumulated in PSUM (+bv per partition,
    exact because softmax rows sum to 1); output projection vs Wo row-slice
    (+bo/4 so the host-side sum of 4 partials reconstructs bo).
"""

import numpy as np

import concourse.bass as bass
import concourse.mybir as mybir
import concourse.tile as tile
from concourse import bacc
from concourse.bass_utils import run_bass_kernel_spmd
from concourse.masks import make_identity

F32 = mybir.dt.float32
BF16 = mybir.dt.bfloat16
RED_DT = BF16  # dtype of tanh output + av weights for the d-reduction matmul
AF = mybir.ActivationFunctionType

B, S, D, H = 2, 512, 512, 8
HD = D // H  # 64
HP = 2  # heads per core
NCORES = 8
G = S // 64  # score groups of 64 queries -> 8

# per 64-query group: DVE broadcast-add blocks cover the first 56 queries,
# the last 8 run the fused ACTIVATE(bias) path directly on ScalarE
DVE_SPANS = (11, 11, 11, 11, 11)
FUSED_Q = 9
assert sum(DVE_SPANS) + FUSED_Q == 64


def build_nc():
    nc = bacc.Bacc("TRN2", target_bir_lowering=False, debug=False, num_devices=NCORES)

    # ---- DRAM I/O (per-core shards; same names on every core) ----
    xqT = nc.dram_tensor("xqT", [D, S], F32, kind="ExternalInput")
    xkT = nc.dram_tensor("xkT", [D, S], F32, kind="ExternalInput")
    xvT = nc.dram_tensor("xvT", [D, S], F32, kind="ExternalInput")
    wq2 = nc.dram_tensor("wq2", [D, 128], F32, kind="ExternalInput")
    wk2 = nc.dram_tensor("wk2", [D, 128], F32, kind="ExternalInput")
    wv2 = nc.dram_tensor("wv2", [D, 128], F32, kind="ExternalInput")
    wo2 = nc.dram_tensor("wo2", [128, D], F32, kind="ExternalInput")
    bq2 = nc.dram_tensor("bq2", [128, 1], F32, kind="ExternalInput")
    bk2 = nc.dram_tensor("bk2", [128, 1], F32, kind="ExternalInput")
    bv2 = nc.dram_tensor("bv2", [128, 1], F32, kind="ExternalInput")
    bo4 = nc.dram_tensor("bo4", [1, D], F32, kind="ExternalInput")
    aq = nc.dram_tensor("aq", [HD, HD], F32, kind="ExternalInput")
    ak = nc.dram_tensor("ak", [HD, HD], F32, kind="ExternalInput")
    av2 = nc.dram_tensor("av2", [HD, 1], F32, kind="ExternalInput")
    attn_out = nc.dram_tensor("attn_out", [2 * S, S], F32, kind="ExternalOutput")
    out_part = nc.dram_tensor("out_part", [S, D], F32, kind="ExternalOutput")

    with tile.TileContext(nc) as tc:
        with (
            tc.tile_pool(name="const", bufs=1) as const,
            tc.tile_pool(name="tpool", bufs=2) as tpool,
            tc.tile_pool(name="apool", bufs=2) as apool,
            tc.tile_pool(name="stats", bufs=4) as stats,
        ):
            # ---------- load inputs ----------
            xq_t = []
            xk_t = []
            xv_t = []
            for m in range(4):
                t = const.tile([128, S], F32, name=f"xq_{m}")
                nc.sync.dma_start(t[:], xqT[m * 128 : (m + 1) * 128, :])
                xq_t.append(t)
                t = const.tile([128, S], F32, name=f"xk_{m}")
                nc.sync.dma_start(t[:], xkT[m * 128 : (m + 1) * 128, :])
                xk_t.append(t)
                t = const.tile([128, S], F32, name=f"xv_{m}")
                nc.sync.dma_start(t[:], xvT[m * 128 : (m + 1) * 128, :])
                xv_t.append(t)
            wq_t = []
            wk_t = []
            wv_t = []
            for m in range(4):
                t = const.tile([128, 128], F32, name=f"wq_{m}")
                nc.sync.dma_start(t[:], wq2[m * 128 : (m + 1) * 128, :])
                wq_t.append(t)
                t = const.tile([128, 128], F32, name=f"wk_{m}")
                nc.sync.dma_start(t[:], wk2[m * 128 : (m + 1) * 128, :])
                wk_t.append(t)
                t = const.tile([128, 128], F32, name=f"wv_{m}")
                nc.sync.dma_start(t[:], wv2[m * 128 : (m + 1) * 128, :])
                wv_t.append(t)
            wo_t = const.tile([128, D], F32, name="wo_t")
            nc.sync.dma_start(wo_t[:], wo2[:, :])
            bq_t = const.tile([128, 1], F32, name="bq_t")
            nc.sync.dma_start(bq_t[:], bq2[:, :])
            bk_t = const.tile([128, 1], F32, name="bk_t")
            nc.sync.dma_start(bk_t[:], bk2[:, :])
            bv_t = const.tile([128, 1], F32, name="bv_t")
            nc.sync.dma_start(bv_t[:], bv2[:, :])
            # bo/4 replicated across partitions via stride-0 DMA
            bo_rep = const.tile([128, D], F32, name="bo_rep")
            bo_bcast = bass.AP(tensor=bo4.ap().tensor, offset=0, ap=[[0, 128], [1, D]])
            nc.sync.dma_start(bo_rep[:], bo_bcast)

            # block-diagonal Aq/Ak [128, 128]
            aq2 = const.tile([128, 128], F32, name="aq2")
            nc.vector.memset(aq2[:], 0.0)
            nc.sync.dma_start(aq2[0:HD, 0:HD], aq[:, :])
            nc.sync.dma_start(aq2[HD:128, HD:128], aq[:, :])
            ak2 = const.tile([128, 128], F32, name="ak2")
            nc.vector.memset(ak2[:], 0.0)
            nc.sync.dma_start(ak2[0:HD, 0:HD], ak[:, :])
            nc.sync.dma_start(ak2[HD:128, HD:128], ak[:, :])

            # shifted-av buffer: col 128 = av on head0 rows, col 129 on head1
            av_sb = const.tile([128, 1], F32, name="av_sb")
            nc.sync.dma_start(av_sb[0:HD, :], av2[:, :])
            nc.sync.dma_start(av_sb[HD:128, :], av2[:, :])
            avb = const.tile([128, 256], RED_DT, name="avb")
            nc.vector.memset(avb[:], 0.0)
            nc.vector.tensor_copy(avb[0:HD, 128:129], av_sb[0:HD, :])
            nc.vector.tensor_copy(avb[HD:128, 129:130], av_sb[HD:128, :])

            ident = const.tile([128, 128], F32, name="ident")
            make_identity(nc, ident[:])

            # ---------- projections (own PSUM phase) ----------
            qat2 = const.tile([128, S], F32, name="qat2")
            kat2 = const.tile([128, S], F32, name="kat2")
            v_t = []
            with tc.tile_pool(name="ps_pro", bufs=2, space="PSUM") as ps_pro:
                for x_t, w_t, b_t, a2, outT in (
                    (xq_t, wq_t, bq_t, aq2, qat2),
                    (xk_t, wk_t, bk_t, ak2, kat2),
                ):
                    pp = ps_pro.tile([128, S], F32, tag="mm", name="pp")
                    for m in range(4):
                        nc.tensor.matmul(
                            pp[:], w_t[m][:], x_t[m][:], start=(m == 0), stop=(m == 3)
                        )
                    pb = const.tile([128, S], F32, name="pb")
                    nc.vector.tensor_scalar_add(pb[:], pp[:], b_t[:])
                    pa = ps_pro.tile([128, S], F32, tag="mm", name="pa")
                    nc.tensor.matmul(pa[:], a2[:], pb[:], start=True, stop=True)
                    nc.vector.tensor_copy(outT[:], pa[:])

                # v [k, (h,hd)] as 4 tiles of [128, 128]
                for kc in range(4):
                    pv = ps_pro.tile([128, S], F32, tag="mm", name="pv")
                    for m in range(4):
                        nc.tensor.matmul(
                            pv[:, 0:128],
                            xv_t[m][:, kc * 128 : (kc + 1) * 128],
                            wv_t[m][:],
                            start=(m == 0),
                            stop=(m == 3),
                        )
                    vt = const.tile([128, 128], F32, name=f"v_{kc}")
                    nc.vector.tensor_copy(vt[:], pv[:, 0:128])
                    v_t.append(vt)

            # attnT chunks: [k-chunk partitions, (group, query, head) columns]
            attnT = [
                const.tile([128, 2 * S], F32, name=f"attnT_{c}") for c in range(4)
            ]

            # ---------- main loop: scores + softmax + transpose ----------
            with (
                tc.tile_pool(name="ps_sc", bufs=2, space="PSUM") as ps_sc,
                tc.tile_pool(name="ps_tp", bufs=3, space="PSUM") as ps_tp,
            ):
                for g in range(G):
                    sc_ps = ps_sc.tile([128, S], F32, tag="sc", name="sc_ps")

                    def emit_red(i, rhs):
                        if i == 0:
                            # full width: start=True must clear the whole bank
                            # (zero lhsT columns write zeros to rows 2..127)
                            nc.tensor.matmul(
                                sc_ps[:, :], avb[:, 128:256], rhs,
                                start=True, stop=False, skip_group_check=True,
                            )
                        else:
                            nc.tensor.matmul(
                                sc_ps[0 : 2 * i + 2, :],
                                avb[:, 128 - 2 * i : 130],
                                rhs,
                                start=False,
                                stop=(i == 63),
                                skip_group_check=True,
                            )

                    pending = None  # reductions for the previous block
                    i0 = 0
                    # DVE broadcast-add blocks + big-FD tanh
                    for bs in DVE_SPANS:
                        q0 = g * 64 + i0
                        tpre = tpool.tile([128, bs, S], F32, tag="tpre", name="tpre")
                        in0 = (
                            qat2[:, q0 : q0 + bs]
                            .unsqueeze(2)
                            .broadcast_to([128, bs, S])
                        )
                        in1 = kat2[:].unsqueeze(1).broadcast_to([128, bs, S])
                        nc.vector.tensor_add(tpre[:, 0:bs, :], in0, in1)
                        td = tpool.tile([128, bs, S], RED_DT, tag="td", name="td")
                        nc.scalar.activation(td[:, 0:bs, :], tpre[:, 0:bs, :], AF.Tanh)
                        if pending is not None:
                            pending()
                        pending = (
                            lambda td=td, i0=i0, bs=bs: [
                                emit_red(i0 + j, td[:, j, :]) for j in range(bs)
                            ]
                        )
                        i0 += bs
                    pending()
                    # fused ACTIVATE(bias) path for the last FUSED_Q queries
                    for j in range(FUSED_Q):
                        i = 64 - FUSED_Q + j
                        q = g * 64 + i
                        tt = tpool.tile([128, S], RED_DT, tag="tt", name="tt",
                                        bufs=3)
                        nc.scalar.activation(
                            tt[:], kat2[:], AF.Tanh, bias=qat2[:, q : q + 1]
                        )
                        emit_red(i, tt[:])

                    # softmax over the 512 free-dim entries of each (q, h) row
                    mx = stats.tile([128, 1], F32, tag="mx", name="mx")
                    nc.vector.tensor_reduce(
                        mx[:], sc_ps[:], axis=mybir.AxisListType.X,
                        op=mybir.AluOpType.max, negate=True,
                    )
                    esum = stats.tile([128, 1], F32, tag="esum", name="esum")
                    attn_e = apool.tile([128, S], F32, tag="attn_e", name="attn_e")
                    nc.scalar.activation(
                        attn_e[:], sc_ps[:], AF.Exp, bias=mx[:], accum_out=esum[:]
                    )
                    rec = stats.tile([128, 1], F32, tag="rec", name="rec")
                    nc.vector.reciprocal(rec[:], esum[:])
                    attn_n = apool.tile([128, S], F32, tag="attn_n", name="attn_n")
                    nc.vector.tensor_scalar_mul(attn_n[:], attn_e[:], rec[:])
                    nc.sync.dma_start(attn_out[g * 128 : (g + 1) * 128, :], attn_n[:])
                    for c in range(4):
                        tp = ps_tp.tile([128, 128], F32, tag="tp", name="tp")
                        nc.tensor.transpose(
                            tp[:], attn_n[:, c * 128 : (c + 1) * 128], ident[:]
                        )
                        nc.vector.tensor_copy(
                            attnT[c][:, g * 128 : (g + 1) * 128], tp[:]
                        )

            # ---------- context + output projection (own PSUM phase) ----------
            with tc.tile_pool(name="ps_epi", bufs=2, space="PSUM") as ps_epi:
                ctx_ps = ps_epi.tile([128, S], F32, tag="mm", name="ctx_ps")
                for h in range(HP):
                    for c in range(4):
                        rhs = attnT[c][:].rearrange(
                            "p (g i h) -> p g i h", g=G, i=64, h=HP
                        )[:, :, :, h]
                        nc.tensor.matmul(
                            ctx_ps[h * HD : (h + 1) * HD, :],
                            v_t[c][:, h * HD : (h + 1) * HD],
                            rhs,
                            start=(c == 0),
                            stop=(c == 3),
                            skip_group_check=True,
                        )
                ctxT = const.tile([128, S], F32, name="ctxT")
                nc.vector.tensor_scalar_add(ctxT[:], ctx_ps[:], bv_t[:])

                for sc in range(4):
                    op_ps = ps_epi.tile([128, S], F32, tag="mm", name="op_ps")
                    nc.tensor.matmul(
                        op_ps[:], ctxT[:, sc * 128 : (sc + 1) * 128], wo_t[:],
                        start=True, stop=True,
                    )
                    ob = apool.tile([128, S], F32, tag="ob", name="ob")
                    nc.vector.tensor_add(ob[:], op_ps[:], bo_rep[:])
                    nc.sync.dma_start(out_part[sc * 128 : (sc + 1) * 128, :], ob[:])

    nc.compile()
    return nc


_NC_CACHE = None


def _get_nc():
    global _NC_CACHE
    if _NC_CACHE is None:
        _NC_CACHE = build_nc()
    return _NC_CACHE


def _prep_core_inputs(c, query, key_, value, Wq, bq, Wk, bk, Wv, bv, Wo, bo, Aq, Ak, av):
    b = c // 4
    hp = c % 4
    cols = slice(hp * 128, hp * 128 + 128)
    cc = np.ascontiguousarray
    return {
        "xqT": cc(query[b].T),
        "xkT": cc(key_[b].T),
        "xvT": cc(value[b].T),
        "wq2": cc(Wq[:, cols]),
        "wk2": cc(Wk[:, cols]),
        "wv2": cc(Wv[:, cols]),
        "wo2": cc(Wo[cols, :]),
        "bq2": cc(bq[cols][:, None]),
        "bk2": cc(bk[cols][:, None]),
        "bv2": cc(bv[cols][:, None]),
        "bo4": cc((bo * 0.25)[None, :]),
        "aq": cc(Aq),
        "ak": cc(Ak),
        "av2": cc(av[:, None]),
    }


def kernel(**inputs):
    f = lambda name: np.asarray(inputs[name], dtype=np.float32)
    args = (
        f("query"), f("key_"), f("value"),
        f("Wq"), f("bq"), f("Wk"), f("bk"), f("Wv"), f("bv"),
        f("Wo"), f("bo"), f("Aq"), f("Ak"), f("av"),
    )
    nc = _get_nc()
    in_maps = [_prep_core_inputs(c, *args) for c in range(NCORES)]
    res = run_bass_kernel_spmd(nc, in_maps, core_ids=list(range(NCORES)))
    results = res.results

    attn = np.empty((B, H, S, S), dtype=np.float32)
    out = np.zeros((B, S, D), dtype=np.float32)
    for c in range(NCORES):
        b = c // 4
        hp = c % 4
        a = results[c]["attn_out"]  # [1024, 512] rows = (g, i, h) interleaved
        a = a.reshape(G, 64, HP, S).transpose(2, 0, 1, 3).reshape(HP, S, S)
        attn[b, 2 * hp : 2 * hp + 2] = a
        out[b] += results[c]["out_part"]
    return out, attn


# revision 11
# speedup vs baseline: 1.2792x; 1.0481x over previous
"""Bass/Trainium2 kernel for additive (Bahdanau-style) multi-head attention.

Reference computation (B=2, S=512, D=512, H=8, HD=64):
    q = heads(query @ Wq + bq); k = heads(key_ @ Wk + bk); v = heads(value @ Wv + bv)
    scores[b,h,i,j] = sum_d tanh((q @ Aq)[b,h,i,d] + (k @ Ak)[b,h,j,d]) * av[d]
    attn = softmax(scores, -1); ctx = attn @ v; out = merge(ctx) @ Wo + bo
    returns (out, attn)

Sharding: 8 cores; core c handles batch b = c // 4 and head pair
h0 = 2*(c % 4), h0+1.  Each core computes its two heads' attention and a
partial output projection; the host sums the 4 partial outputs per batch.

Per-core device plan (v4):
  - q/k projection chain in bf16 (host supplies bf16 copies) to get
    qAT2/kAT2 [128=(h,hd), 512 tokens] fast; v / Wo projections in fp32.
  - The [dd, q, k] pre-tanh sum is built two ways, balanced across engines:
      * DVE blocks (~11 q): one tensor_tensor add with stride-0 broadcast
        APs -> [128, bs*512] fp32, then one big ACTIVATE Tanh -> bf16.
      * fused path (last 10 q of each 64-group): ACTIVATE Tanh with
        per-partition bias does add+tanh in one [128, 512] instruction.
  - TensorE reduces over d with av (bf16): shifted-view lhsT accumulates
    query q into PSUM rows (2*(q%64), 2*(q%64)+1) of a [128, 512] score
    bank; lhsT width trimmed to 2*(q%64)+2 (full width at i=0 to clear).
  - Softmax per row: DVE max (negated), ScalarE Exp(bias=-max, accum_out)
    reading PSUM, DVE reciprocal + per-partition multiply.
  - attn rows DMA out interleaved; host de-interleaves.
  - PE transpose of attn tiles -> attnT; ctx^T accumulated incrementally
    per group into a persistent PSUM bank; output projection chunks are
    emitted as soon as their two groups are done (+bv per partition; +bo/4
    so the host-side sum of 4 partials reconstructs bo).
"""

import numpy as np

import concourse.bass as bass
import concourse.mybir as mybir
import concourse.tile as tile
from concourse import bacc
from concourse.bass_utils import run_bass_kernel_spmd
from concourse.masks import make_identity

F32 = mybir.dt.float32
BF16 = mybir.dt.bfloat16
RED_DT = BF16
AF = mybir.ActivationFunctionType

B, S, D, H = 2, 512, 512, 8
HD = D // H  # 64
HP = 2
NCORES = 8
G = S // 64  # 8 groups of 64 queries

DVE_SPANS = (11, 11, 11, 11, 10)
FUSED_Q = 10
assert sum(DVE_SPANS) + FUSED_Q == 64


def build_nc():
    nc = bacc.Bacc("TRN2", target_bir_lowering=False, debug=False, num_devices=NCORES)

    # ---- DRAM I/O (per-core shards; same names on every core) ----
    xqTb = nc.dram_tensor("xqTb", [D, S], BF16, kind="ExternalInput")
    xkTb = nc.dram_tensor("xkTb", [D, S], BF16, kind="ExternalInput")
    xvT = nc.dram_tensor("xvT", [D, S], F32, kind="ExternalInput")
    wq2b = nc.dram_tensor("wq2b", [D, 128], BF16, kind="ExternalInput")
    wk2b = nc.dram_tensor("wk2b", [D, 128], BF16, kind="ExternalInput")
    wv2 = nc.dram_tensor("wv2", [D, 128], F32, kind="ExternalInput")
    wo2 = nc.dram_tensor("wo2", [128, D], F32, kind="ExternalInput")
    bq2 = nc.dram_tensor("bq2", [128, 1], F32, kind="ExternalInput")
    bk2 = nc.dram_tensor("bk2", [128, 1], F32, kind="ExternalInput")
    bv2 = nc.dram_tensor("bv2", [128, 1], F32, kind="ExternalInput")
    bo4 = nc.dram_tensor("bo4", [1, D], F32, kind="ExternalInput")
    aqb = nc.dram_tensor("aqb", [HD, HD], BF16, kind="ExternalInput")
    akb = nc.dram_tensor("akb", [HD, HD], BF16, kind="ExternalInput")
    av2 = nc.dram_tensor("av2", [HD, 1], F32, kind="ExternalInput")
    attn_out = nc.dram_tensor("attn_out", [2 * S, S], F32, kind="ExternalOutput")
    out_part = nc.dram_tensor("out_part", [S, D], F32, kind="ExternalOutput")

    with tile.TileContext(nc) as tc:
        with (
            tc.tile_pool(name="const", bufs=1) as const,
            tc.tile_pool(name="tpool", bufs=2) as tpool,
            tc.tile_pool(name="apool", bufs=2) as apool,
            tc.tile_pool(name="stats", bufs=4) as stats,
            tc.tile_pool(name="ps_sc", bufs=2, space="PSUM") as ps_sc,
            tc.tile_pool(name="ps_tp", bufs=2, space="PSUM") as ps_tp,
            tc.tile_pool(name="ps_cx", bufs=1, space="PSUM") as ps_cx,
            tc.tile_pool(name="ps_mm", bufs=2, space="PSUM") as ps_mm,
        ):
            # ---------- q/k projection critical chain (bf16) ----------
            xqb_t = []
            xkb_t = []
            wqb_t = []
            wkb_t = []
            for m in range(4):
                t = const.tile([128, S], BF16, name=f"xqb_{m}")
                nc.sync.dma_start(t[:], xqTb[m * 128 : (m + 1) * 128, :])
                xqb_t.append(t)
                t = const.tile([128, 128], BF16, name=f"wqb_{m}")
                nc.sync.dma_start(t[:], wq2b[m * 128 : (m + 1) * 128, :])
                wqb_t.append(t)
            for m in range(4):
                t = const.tile([128, S], BF16, name=f"xkb_{m}")
                nc.sync.dma_start(t[:], xkTb[m * 128 : (m + 1) * 128, :])
                xkb_t.append(t)
                t = const.tile([128, 128], BF16, name=f"wkb_{m}")
                nc.sync.dma_start(t[:], wk2b[m * 128 : (m + 1) * 128, :])
                wkb_t.append(t)
            bq_t = const.tile([128, 1], F32, name="bq_t")
            nc.sync.dma_start(bq_t[:], bq2[:, :])
            bk_t = const.tile([128, 1], F32, name="bk_t")
            nc.sync.dma_start(bk_t[:], bk2[:, :])
            # block-diagonal Aq/Ak [128, 128] bf16
            aq2 = const.tile([128, 128], BF16, name="aq2")
            nc.vector.memset(aq2[:], 0.0)
            nc.sync.dma_start(aq2[0:HD, 0:HD], aqb[:, :])
            nc.sync.dma_start(aq2[HD:128, HD:128], aqb[:, :])
            ak2 = const.tile([128, 128], BF16, name="ak2")
            nc.vector.memset(ak2[:], 0.0)
            nc.sync.dma_start(ak2[0:HD, 0:HD], akb[:, :])
            nc.sync.dma_start(ak2[HD:128, HD:128], akb[:, :])

            qat2 = const.tile([128, S], F32, name="qat2")
            kat2 = const.tile([128, S], F32, name="kat2")
            for xb_t, wb_t, b_t, a2, outT in (
                (xqb_t, wqb_t, bq_t, aq2, qat2),
                (xkb_t, wkb_t, bk_t, ak2, kat2),
            ):
                pp = ps_mm.tile([128, S], F32, tag="mm", name="pp")
                for m in range(4):
                    nc.tensor.matmul(
                        pp[:], wb_t[m][:], xb_t[m][:], start=(m == 0), stop=(m == 3)
                    )
                pb = const.tile([128, S], BF16, name="pb")
                nc.vector.tensor_scalar_add(pb[:], pp[:], b_t[:])
                pa = ps_mm.tile([128, S], F32, tag="mm", name="pa")
                nc.tensor.matmul(pa[:], a2[:], pb[:], start=True, stop=True)
                nc.vector.tensor_copy(outT[:], pa[:])

            # shifted-av buffer: col 128 = av on head0 rows, col 129 on head1
            av_sb = const.tile([128, 1], F32, name="av_sb")
            nc.sync.dma_start(av_sb[0:HD, :], av2[:, :])
            nc.sync.dma_start(av_sb[HD:128, :], av2[:, :])
            avb = const.tile([128, 256], RED_DT, name="avb")
            nc.vector.memset(avb[:], 0.0)
            nc.vector.tensor_copy(avb[0:HD, 128:129], av_sb[0:HD, :])
            nc.vector.tensor_copy(avb[HD:128, 129:130], av_sb[HD:128, :])

            # ---------- non-critical loads: v projection, Wo, biases ----------
            xv_t = []
            wv_t = []
            for m in range(4):
                t = const.tile([128, S], F32, name=f"xv_{m}")
                nc.sync.dma_start(t[:], xvT[m * 128 : (m + 1) * 128, :])
                xv_t.append(t)
                t = const.tile([128, 128], F32, name=f"wv_{m}")
                nc.sync.dma_start(t[:], wv2[m * 128 : (m + 1) * 128, :])
                wv_t.append(t)
            wo_t = const.tile([128, D], F32, name="wo_t")
            nc.sync.dma_start(wo_t[:], wo2[:, :])
            bv_t = const.tile([128, 1], F32, name="bv_t")
            nc.sync.dma_start(bv_t[:], bv2[:, :])
            bo_rep = const.tile([128, D], F32, name="bo_rep")
            bo_bcast = bass.AP(tensor=bo4.ap().tensor, offset=0, ap=[[0, 128], [1, D]])
            nc.sync.dma_start(bo_rep[:], bo_bcast)

            ident = const.tile([128, 128], F32, name="ident")
            make_identity(nc, ident[:])

            v_t = []
            for kc in range(4):
                pv = ps_mm.tile([128, S], F32, tag="mm", name="pv")
                for m in range(4):
                    nc.tensor.matmul(
                        pv[:, 0:128],
                        xv_t[m][:, kc * 128 : (kc + 1) * 128],
                        wv_t[m][:],
                        start=(m == 0),
                        stop=(m == 3),
                    )
                vt = const.tile([128, 128], F32, name=f"v_{kc}")
                nc.vector.tensor_copy(vt[:], pv[:, 0:128])
                v_t.append(vt)

            attnT = [
                const.tile([128, 2 * S], F32, name=f"attnT_{c}") for c in range(4)
            ]
            ctx_ps = ps_cx.tile([128, S], F32, tag="cx", name="ctx_ps")

            # ---------- main loop ----------
            for g in range(G):
                sc_ps = ps_sc.tile([128, S], F32, tag="sc", name="sc_ps")

                def emit_red(i, rhs):
                    if i == 0:
                        nc.tensor.matmul(
                            sc_ps[:, :], avb[:, 128:256], rhs,
                            start=True, stop=False, skip_group_check=True,
                        )
                    else:
                        nc.tensor.matmul(
                            sc_ps[0 : 2 * i + 2, :],
                            avb[:, 128 - 2 * i : 130],
                            rhs,
                            start=False,
                            stop=(i == 63),
                            skip_group_check=True,
                        )

                pending = None
                i0 = 0
                for bs in DVE_SPANS:
                    q0 = g * 64 + i0
                    tpre = tpool.tile([128, bs, S], F32, tag="tpre", name="tpre")
                    in0 = (
                        qat2[:, q0 : q0 + bs].unsqueeze(2).broadcast_to([128, bs, S])
                    )
                    in1 = kat2[:].unsqueeze(1).broadcast_to([128, bs, S])
                    nc.vector.tensor_add(tpre[:, 0:bs, :], in0, in1)
                    td = tpool.tile([128, bs, S], RED_DT, tag="td", name="td")
                    nc.scalar.activation(td[:, 0:bs, :], tpre[:, 0:bs, :], AF.Tanh)
                    if pending is not None:
                        pending()
                    pending = (
                        lambda td=td, i0=i0, bs=bs: [
                            emit_red(i0 + j, td[:, j, :]) for j in range(bs)
                        ]
                    )
                    i0 += bs
                pending()
                for j in range(FUSED_Q):
                    i = 64 - FUSED_Q + j
                    q = g * 64 + i
                    tt = tpool.tile([128, S], RED_DT, tag="tt", name="tt", bufs=3)
                    nc.scalar.activation(
                        tt[:], kat2[:], AF.Tanh, bias=qat2[:, q : q + 1]
                    )
                    emit_red(i, tt[:])

                # softmax per (q, h) row
                mx = stats.tile([128, 1], F32, tag="mx", name="mx")
                nc.vector.tensor_reduce(
                    mx[:], sc_ps[:], axis=mybir.AxisListType.X,
                    op=mybir.AluOpType.max, negate=True,
                )
                esum = stats.tile([128, 1], F32, tag="esum", name="esum")
                attn_e = apool.tile([128, S], F32, tag="attn_e", name="attn_e")
                nc.scalar.activation(
                    attn_e[:], sc_ps[:], AF.Exp, bias=mx[:], accum_out=esum[:]
                )
                rec = stats.tile([128, 1], F32, tag="rec", name="rec")
                nc.vector.reciprocal(rec[:], esum[:])
                attn_n = apool.tile([128, S], F32, tag="attn_n", name="attn_n")
                nc.vector.tensor_scalar_mul(attn_n[:], attn_e[:], rec[:])
                nc.sync.dma_start(attn_out[g * 128 : (g + 1) * 128, :], attn_n[:])

                # transpose attn tiles into attnT
                for c in range(4):
                    tp = ps_tp.tile([128, 128], F32, tag="tp", name="tp")
                    nc.tensor.transpose(
                        tp[:], attn_n[:, c * 128 : (c + 1) * 128], ident[:]
                    )
                    nc.vector.tensor_copy(attnT[c][:, g * 128 : (g + 1) * 128], tp[:])

                # incremental ctx^T for this group's query columns
                for h in range(HP):
                    for c in range(4):
                        rhs = attnT[c][:].rearrange(
                            "p (g i h) -> p g i h", g=G, i=64, h=HP
                        )[:, g, :, h]
                        nc.tensor.matmul(
                            ctx_ps[h * HD : (h + 1) * HD, g * 64 : (g + 1) * 64],
                            v_t[c][:, h * HD : (h + 1) * HD],
                            rhs,
                            start=(c == 0),
                            stop=(c == 3),
                            skip_group_check=True,
                        )

                # emit output-projection chunk once its two groups are done
                if g % 2 == 1:
                    sc = g // 2
                    ctxT_c = apool.tile([128, 128], F32, tag="ctxT", name="ctxT_c")
                    nc.vector.tensor_scalar_add(
                        ctxT_c[:], ctx_ps[:, sc * 128 : (sc + 1) * 128], bv_t[:]
                    )
                    op_ps = ps_mm.tile([128, S], F32, tag="mm", name="op_ps")
                    nc.tensor.matmul(
                        op_ps[:], ctxT_c[:], wo_t[:], start=True, stop=True
                    )
                    ob = apool.tile([128, S], F32, tag="ob", name="ob")
                    nc.vector.tensor_add(ob[:], op_ps[:], bo_rep[:])
                    nc.sync.dma_start(out_part[sc * 128 : (sc + 1) * 128, :], ob[:])

    nc.compile()
    return nc


_NC_CACHE = None


def _get_nc():
    global _NC_CACHE
    if _NC_CACHE is None:
        _NC_CACHE = build_nc()
    return _NC_CACHE


def _prep_core_inputs(c, query, key_, value, Wq, bq, Wk, bk, Wv, bv, Wo, bo, Aq, Ak, av):
    b = c // 4
    hp = c % 4
    cols = slice(hp * 128, hp * 128 + 128)
    cc = np.ascontiguousarray
    import ml_dtypes

    bf = lambda x: np.ascontiguousarray(x, dtype=ml_dtypes.bfloat16)
    return {
        "xqTb": bf(query[b].T),
        "xkTb": bf(key_[b].T),
        "xvT": cc(value[b].T),
        "wq2b": bf(Wq[:, cols]),
        "wk2b": bf(Wk[:, cols]),
        "wv2": cc(Wv[:, cols]),
        "wo2": cc(Wo[cols, :]),
        "bq2": cc(bq[cols][:, None]),
        "bk2": cc(bk[cols][:, None]),
        "bv2": cc(bv[cols][:, None]),
        "bo4": cc((bo * 0.25)[None, :]),
        "aqb": bf(Aq),
        "akb": bf(Ak),
        "av2": cc(av[:, None]),
    }


def kernel(**inputs):
    f = lambda name: np.asarray(inputs[name], dtype=np.float32)
    args = (
        f("query"), f("key_"), f("value"),
        f("Wq"), f("bq"), f("Wk"), f("bk"), f("Wv"), f("bv"),
        f("Wo"), f("bo"), f("Aq"), f("Ak"), f("av"),
    )
    nc = _get_nc()
    in_maps = [_prep_core_inputs(c, *args) for c in range(NCORES)]
    res = run_bass_kernel_spmd(nc, in_maps, core_ids=list(range(NCORES)))
    results = res.results

    attn = np.empty((B, H, S, S), dtype=np.float32)
    out = np.zeros((B, S, D), dtype=np.float32)
    for c in range(NCORES):
        b = c // 4
        hp = c % 4
        a = results[c]["attn_out"]  # [1024, 512] rows = (g, i, h) interleaved
        a = a.reshape(G, 64, HP, S).transpose(2, 0, 1, 3).reshape(HP, S, S)
        attn[b, 2 * hp : 2 * hp + 2] = a
        out[b] += results[c]["out_part"]
    return out, attn


# revision 12
# speedup vs baseline: 1.3128x; 1.0263x over previous
"""Bass/Trainium2 kernel for additive (Bahdanau-style) multi-head attention.

Reference computation (B=2, S=512, D=512, H=8, HD=64):
    q = heads(query @ Wq + bq); k = heads(key_ @ Wk + bk); v = heads(value @ Wv + bv)
    scores[b,h,i,j] = sum_d tanh((q @ Aq)[b,h,i,d] + (k @ Ak)[b,h,j,d]) * av[d]
    attn = softmax(scores, -1); ctx = attn @ v; out = merge(ctx) @ Wo + bo
    returns (out, attn)

Sharding: 8 cores; core c handles batch b = c // 4 and head pair
h0 = 2*(c % 4), h0+1.  Each core computes its two heads' attention and a
partial output projection; the host sums the 4 partial outputs per batch.

Per-core device plan (v4):
  - q/k projection chain in bf16 (host supplies bf16 copies) to get
    qAT2/kAT2 [128=(h,hd), 512 tokens] fast; v / Wo projections in fp32.
  - The [dd, q, k] pre-tanh sum is built two ways, balanced across engines:
      * DVE blocks (~11 q): one tensor_tensor add with stride-0 broadcast
        APs -> [128, bs*512] fp32, then one big ACTIVATE Tanh -> bf16.
      * fused path (last 10 q of each 64-group): ACTIVATE Tanh with
        per-partition bias does add+tanh in one [128, 512] instruction.
  - TensorE reduces over d with av (bf16): shifted-view lhsT accumulates
    query q into PSUM rows (2*(q%64), 2*(q%64)+1) of a [128, 512] score
    bank; lhsT width trimmed to 2*(q%64)+2 (full width at i=0 to clear).
  - Softmax per row: DVE max (negated), ScalarE Exp(bias=-max, accum_out)
    reading PSUM, DVE reciprocal + per-partition multiply.
  - attn rows DMA out interleaved; host de-interleaves.
  - PE transpose of attn tiles -> attnT; ctx^T accumulated incrementally
    per group into a persistent PSUM bank; output projection chunks are
    emitted as soon as their two groups are done (+bv per partition; +bo/4
    so the host-side sum of 4 partials reconstructs bo).
"""

import numpy as np

import concourse.bass as bass
import concourse.mybir as mybir
import concourse.tile as tile
from concourse import bacc
from concourse.bass_utils import run_bass_kernel_spmd
from concourse.masks import make_identity

F32 = mybir.dt.float32
BF16 = mybir.dt.bfloat16
RED_DT = BF16
AF = mybir.ActivationFunctionType

B, S, D, H = 2, 512, 512, 8
HD = D // H  # 64
HP = 2
NCORES = 8
G = S // 64  # 8 groups of 64 queries

DVE_SPANS = (11, 11, 11, 11, 10)
FUSED_Q = 10
assert sum(DVE_SPANS) + FUSED_Q == 64


def build_nc():
    nc = bacc.Bacc("TRN2", target_bir_lowering=False, debug=False, num_devices=NCORES)

    # ---- DRAM I/O (per-core shards; same names on every core) ----
    xqTb = nc.dram_tensor("xqTb", [D, S], BF16, kind="ExternalInput")
    xkTb = nc.dram_tensor("xkTb", [D, S], BF16, kind="ExternalInput")
    xvT = nc.dram_tensor("xvT", [D, S], F32, kind="ExternalInput")
    wq2b = nc.dram_tensor("wq2b", [D, 128], BF16, kind="ExternalInput")
    wk2b = nc.dram_tensor("wk2b", [D, 128], BF16, kind="ExternalInput")
    wv2 = nc.dram_tensor("wv2", [D, 128], F32, kind="ExternalInput")
    wo2 = nc.dram_tensor("wo2", [128, D], F32, kind="ExternalInput")
    bq2 = nc.dram_tensor("bq2", [128, 1], F32, kind="ExternalInput")
    bk2 = nc.dram_tensor("bk2", [128, 1], F32, kind="ExternalInput")
    bv2 = nc.dram_tensor("bv2", [128, 1], F32, kind="ExternalInput")
    bo4 = nc.dram_tensor("bo4", [1, D], F32, kind="ExternalInput")
    aqb = nc.dram_tensor("aqb", [HD, HD], BF16, kind="ExternalInput")
    akb = nc.dram_tensor("akb", [HD, HD], BF16, kind="ExternalInput")
    av2 = nc.dram_tensor("av2", [HD, 1], F32, kind="ExternalInput")
    attn_out = nc.dram_tensor("attn_out", [2 * S, S], F32, kind="ExternalOutput")
    out_part = nc.dram_tensor("out_part", [S, D], F32, kind="ExternalOutput")

    with tile.TileContext(nc) as tc:
        with (
            tc.tile_pool(name="const", bufs=1) as const,
            tc.tile_pool(name="tpool", bufs=2) as tpool,
            tc.tile_pool(name="apool", bufs=2) as apool,
            tc.tile_pool(name="stats", bufs=4) as stats,
            tc.tile_pool(name="ps_sc", bufs=2, space="PSUM") as ps_sc,
            tc.tile_pool(name="ps_tp", bufs=2, space="PSUM") as ps_tp,
            tc.tile_pool(name="ps_cx", bufs=1, space="PSUM") as ps_cx,
            tc.tile_pool(name="ps_mm", bufs=2, space="PSUM") as ps_mm,
        ):
            # ---------- q/k projection critical chain (bf16) ----------
            xqb = const.tile([128, 4, S], BF16, name="xqb")
            nc.sync.dma_start(xqb[:], xqTb.ap().rearrange("(c p) s -> p c s", p=128))
            wqb = const.tile([128, 4, 128], BF16, name="wqb")
            nc.sync.dma_start(wqb[:], wq2b.ap().rearrange("(c p) s -> p c s", p=128))
            xkb = const.tile([128, 4, S], BF16, name="xkb")
            nc.sync.dma_start(xkb[:], xkTb.ap().rearrange("(c p) s -> p c s", p=128))
            wkb = const.tile([128, 4, 128], BF16, name="wkb")
            nc.sync.dma_start(wkb[:], wk2b.ap().rearrange("(c p) s -> p c s", p=128))
            xqb_t = [xqb[:, m, :] for m in range(4)]
            wqb_t = [wqb[:, m, :] for m in range(4)]
            xkb_t = [xkb[:, m, :] for m in range(4)]
            wkb_t = [wkb[:, m, :] for m in range(4)]
            bq_t = const.tile([128, 1], F32, name="bq_t")
            nc.sync.dma_start(bq_t[:], bq2[:, :])
            bk_t = const.tile([128, 1], F32, name="bk_t")
            nc.sync.dma_start(bk_t[:], bk2[:, :])
            # block-diagonal Aq/Ak [128, 128] bf16
            aq2 = const.tile([128, 128], BF16, name="aq2")
            nc.vector.memset(aq2[:], 0.0)
            nc.sync.dma_start(aq2[0:HD, 0:HD], aqb[:, :])
            nc.sync.dma_start(aq2[HD:128, HD:128], aqb[:, :])
            ak2 = const.tile([128, 128], BF16, name="ak2")
            nc.vector.memset(ak2[:], 0.0)
            nc.sync.dma_start(ak2[0:HD, 0:HD], akb[:, :])
            nc.sync.dma_start(ak2[HD:128, HD:128], akb[:, :])

            qat2 = const.tile([128, S], F32, name="qat2")
            kat2 = const.tile([128, S], F32, name="kat2")
            for xb_t, wb_t, b_t, a2, outT in (
                (xqb_t, wqb_t, bq_t, aq2, qat2),
                (xkb_t, wkb_t, bk_t, ak2, kat2),
            ):
                pp = ps_mm.tile([128, S], F32, tag="mm", name="pp")
                for m in range(4):
                    nc.tensor.matmul(
                        pp[:], wb_t[m][:], xb_t[m][:], start=(m == 0), stop=(m == 3)
                    )
                pb = const.tile([128, S], BF16, name="pb")
                nc.vector.tensor_scalar_add(pb[:], pp[:], b_t[:])
                pa = ps_mm.tile([128, S], F32, tag="mm", name="pa")
                nc.tensor.matmul(pa[:], a2[:], pb[:], start=True, stop=True)
                nc.vector.tensor_copy(outT[:], pa[:])

            # shifted-av buffer: col 128 = av on head0 rows, col 129 on head1
            av_sb = const.tile([128, 1], F32, name="av_sb")
            nc.sync.dma_start(av_sb[0:HD, :], av2[:, :])
            nc.sync.dma_start(av_sb[HD:128, :], av2[:, :])
            avb = const.tile([128, 256], RED_DT, name="avb")
            nc.vector.memset(avb[:], 0.0)
            nc.vector.tensor_copy(avb[0:HD, 128:129], av_sb[0:HD, :])
            nc.vector.tensor_copy(avb[HD:128, 129:130], av_sb[HD:128, :])

            # ---------- non-critical loads: v projection, Wo, biases ----------
            xv = const.tile([128, 4, S], F32, name="xv")
            nc.sync.dma_start(xv[:], xvT.ap().rearrange("(c p) s -> p c s", p=128))
            wv = const.tile([128, 4, 128], F32, name="wv")
            nc.sync.dma_start(wv[:], wv2.ap().rearrange("(c p) s -> p c s", p=128))
            xv_t = [xv[:, m, :] for m in range(4)]
            wv_t = [wv[:, m, :] for m in range(4)]
            wo_t = const.tile([128, D], F32, name="wo_t")
            nc.sync.dma_start(wo_t[:], wo2[:, :])
            bv_t = const.tile([128, 1], F32, name="bv_t")
            nc.sync.dma_start(bv_t[:], bv2[:, :])
            bo_rep = const.tile([128, D], F32, name="bo_rep")
            bo_bcast = bass.AP(tensor=bo4.ap().tensor, offset=0, ap=[[0, 128], [1, D]])
            nc.sync.dma_start(bo_rep[:], bo_bcast)

            ident = const.tile([128, 128], F32, name="ident")
            make_identity(nc, ident[:])

            v_t = []
            for kc in range(4):
                pv = ps_mm.tile([128, S], F32, tag="mm", name="pv")
                for m in range(4):
                    nc.tensor.matmul(
                        pv[:, 0:128],
                        xv_t[m][:, kc * 128 : (kc + 1) * 128],
                        wv_t[m][:],
                        start=(m == 0),
                        stop=(m == 3),
                    )
                vt = const.tile([128, 128], F32, name=f"v_{kc}")
                nc.vector.tensor_copy(vt[:], pv[:, 0:128])
                v_t.append(vt)

            attnT = [
                const.tile([128, 2 * S], F32, name=f"attnT_{c}") for c in range(4)
            ]
            ctx_ps = ps_cx.tile([128, S], F32, tag="cx", name="ctx_ps")

            # ---------- main loop ----------
            for g in range(G):
                sc_ps = ps_sc.tile([128, S], F32, tag="sc", name="sc_ps")

                def emit_red(i, rhs):
                    if i == 0:
                        nc.tensor.matmul(
                            sc_ps[:, :], avb[:, 128:256], rhs,
                            start=True, stop=False, skip_group_check=True,
                        )
                    else:
                        nc.tensor.matmul(
                            sc_ps[0 : 2 * i + 2, :],
                            avb[:, 128 - 2 * i : 130],
                            rhs,
                            start=False,
                            stop=(i == 63),
                            skip_group_check=True,
                        )

                pending = None
                i0 = 0
                for bs in DVE_SPANS:
                    q0 = g * 64 + i0
                    tpre = tpool.tile([128, bs, S], F32, tag="tpre", name="tpre")
                    in0 = (
                        qat2[:, q0 : q0 + bs].unsqueeze(2).broadcast_to([128, bs, S])
                    )
                    in1 = kat2[:].unsqueeze(1).broadcast_to([128, bs, S])
                    nc.vector.tensor_add(tpre[:, 0:bs, :], in0, in1)
                    td = tpool.tile([128, bs, S], RED_DT, tag="td", name="td")
                    nc.scalar.activation(td[:, 0:bs, :], tpre[:, 0:bs, :], AF.Tanh)
                    if pending is not None:
                        pending()
                    pending = (
                        lambda td=td, i0=i0, bs=bs: [
                            emit_red(i0 + j, td[:, j, :]) for j in range(bs)
                        ]
                    )
                    i0 += bs
                pending()
                for j in range(FUSED_Q):
                    i = 64 - FUSED_Q + j
                    q = g * 64 + i
                    tt = tpool.tile([128, S], RED_DT, tag="tt", name="tt", bufs=3)
                    nc.scalar.activation(
                        tt[:], kat2[:], AF.Tanh, bias=qat2[:, q : q + 1]
                    )
                    emit_red(i, tt[:])

                # softmax per (q, h) row
                mx = stats.tile([128, 1], F32, tag="mx", name="mx")
                nc.vector.tensor_reduce(
                    mx[:], sc_ps[:], axis=mybir.AxisListType.X,
                    op=mybir.AluOpType.max, negate=True,
                )
                esum = stats.tile([128, 1], F32, tag="esum", name="esum")
                attn_e = apool.tile([128, S], F32, tag="attn_e", name="attn_e")
                nc.scalar.activation(
                    attn_e[:], sc_ps[:], AF.Exp, bias=mx[:], accum_out=esum[:]
                )
                rec = stats.tile([128, 1], F32, tag="rec", name="rec")
                nc.vector.reciprocal(rec[:], esum[:])
                attn_n = apool.tile([128, S], F32, tag="attn_n", name="attn_n")
                nc.vector.tensor_scalar_mul(attn_n[:], attn_e[:], rec[:])
                nc.sync.dma_start(attn_out[g * 128 : (g + 1) * 128, :], attn_n[:])

                # transpose attn tiles into attnT
                for c in range(4):
                    tp = ps_tp.tile([128, 128], F32, tag="tp", name="tp")
                    nc.tensor.transpose(
                        tp[:], attn_n[:, c * 128 : (c + 1) * 128], ident[:]
                    )
                    nc.vector.tensor_copy(attnT[c][:, g * 128 : (g + 1) * 128], tp[:])

                # incremental ctx^T for this group's query columns
                for h in range(HP):
                    for c in range(4):
                        rhs = attnT[c][:].rearrange(
                            "p (g i h) -> p g i h", g=G, i=64, h=HP
                        )[:, g, :, h]
                        nc.tensor.matmul(
                            ctx_ps[h * HD : (h + 1) * HD, g * 64 : (g + 1) * 64],
                            v_t[c][:, h * HD : (h + 1) * HD],
                            rhs,
                            start=(c == 0),
                            stop=(c == 3),
                            skip_group_check=True,
                        )

                # emit output-projection chunk once its two groups are done
                if g % 2 == 1:
                    sc = g // 2
                    ctxT_c = apool.tile([128, 128], F32, tag="ctxT", name="ctxT_c")
                    nc.vector.tensor_scalar_add(
                        ctxT_c[:], ctx_ps[:, sc * 128 : (sc + 1) * 128], bv_t[:]
                    )
                    op_ps = ps_mm.tile([128, S], F32, tag="mm", name="op_ps")
                    nc.tensor.matmul(
                        op_ps[:], ctxT_c[:], wo_t[:], start=True, stop=True
                    )
                    ob = apool.tile([128, S], F32, tag="ob", name="ob")
                    nc.vector.tensor_add(ob[:], op_ps[:], bo_rep[:])
                    nc.sync.dma_start(out_part[sc * 128 : (sc + 1) * 128, :], ob[:])

    nc.compile()
    return nc


_NC_CACHE = None


def _get_nc():
    global _NC_CACHE
    if _NC_CACHE is None:
        _NC_CACHE = build_nc()
    return _NC_CACHE


def _prep_core_inputs(c, query, key_, value, Wq, bq, Wk, bk, Wv, bv, Wo, bo, Aq, Ak, av):
    b = c // 4
    hp = c % 4
    cols = slice(hp * 128, hp * 128 + 128)
    cc = np.ascontiguousarray
    import ml_dtypes

    bf = lambda x: np.ascontiguousarray(x, dtype=ml_dtypes.bfloat16)
    return {
        "xqTb": bf(query[b].T),
        "xkTb": bf(key_[b].T),
        "xvT": cc(value[b].T),
        "wq2b": bf(Wq[:, cols]),
        "wk2b": bf(Wk[:, cols]),
        "wv2": cc(Wv[:, cols]),
        "wo2": cc(Wo[cols, :]),
        "bq2": cc(bq[cols][:, None]),
        "bk2": cc(bk[cols][:, None]),
        "bv2": cc(bv[cols][:, None]),
        "bo4": cc((bo * 0.25)[None, :]),
        "aqb": bf(Aq),
        "akb": bf(Ak),
        "av2": cc(av[:, None]),
    }


def kernel(**inputs):
    f = lambda name: np.asarray(inputs[name], dtype=np.float32)
    args = (
        f("query"), f("key_"), f("value"),
        f("Wq"), f("bq"), f("Wk"), f("bk"), f("Wv"), f("bv"),
        f("Wo"), f("bo"), f("Aq"), f("Ak"), f("av"),
    )
    nc = _get_nc()
    in_maps = [_prep_core_inputs(c, *args) for c in range(NCORES)]
    res = run_bass_kernel_spmd(nc, in_maps, core_ids=list(range(NCORES)))
    results = res.results

    attn = np.empty((B, H, S, S), dtype=np.float32)
    out = np.zeros((B, S, D), dtype=np.float32)
    for c in range(NCORES):
        b = c // 4
        hp = c % 4
        a = results[c]["attn_out"]  # [1024, 512] rows = (g, i, h) interleaved
        a = a.reshape(G, 64, HP, S).transpose(2, 0, 1, 3).reshape(HP, S, S)
        attn[b, 2 * hp : 2 * hp + 2] = a
        out[b] += results[c]["out_part"]
    return out, attn
